# revision 32
# baseline (speedup 1.0000x reference)
"""Trainium2 Bass kernel for the batched multi-period portfolio QP
(projected subgradient descent, 200 iterations) — v2.

Strategy (per spec sharding hint): B=128 QP instances sharded 16 per core
across 8 NeuronCores; each core solves its 16*12 = 192 independent
128-dim QPs entirely on-chip.

v3 changes vs the v1 baseline:
  - Swapped-operand matvec: since Sigma is symmetric, q_v^T = w_v^T Sigma_v.
    The 1..32-column w weights are the PE stationary operand (cheap
    LDWEIGHTS) and Sigma streams as the moving operand (fp16 streams 2
    cols/cycle), instead of reloading a full 128x128 Sigma weight per
    instance. Measured ~42ns vs ~104ns per matvec.
  - Column-tiled output scatter: instance v lands on PSUM partition v by
    issuing, per 32-column array strip, matmuls in descending weight-width
    order; later (narrower) matmuls overwrite the garbage rows below, so
    each partition's final value comes from its own instance. This puts q
    directly in NATURAL layout (instance on partition), eliminating the
    per-iteration transpose-back of the gradient.
  - The L1 trade-cost term COST*t^T and the constant -mu^T are both
    accumulated into the same PSUM tile by two extra fp16 matmuls per
    group (lhsT = t16 / -mu16, rhs = COST*I / I, start=False), so the
    projection chain starts with a single fused DVE op reading PSUM.
  - Explicit software pipelining: the hardware loop body covers two solver
    iterations with explicit A/B PSUM buffers; the G1 projection of
    iteration k is hoisted under iteration k+1's G0 matvec stream, and
    tr(W0)+cast for k+1 run inside k's G1 stream, so the PE never idles
    (keeps the HAM clock gate at 2.4 GHz).

Per-core algorithm:
  - Precompute Sigma2G[v] = 2*GAMMA * L_v @ L_v^T on the TensorEngine
    (fp32), stored in SBUF as fp16 (host-validated: final rel err ~5e-4).
  - 200 iterations of:
      W_T   = transpose(W)                     (PE, fp32)
      s_T   = sign(W_T - shift_h(W_T))         (ACT Sign; shifts are free-dim
                                                AP offsets in transposed layout)
      t16   = s_T - s_next_T                   (DVE, fp16)
      q     = per-instance w_v^T Sigma_v       (192 swapped matmuls -> natural
                                                PSUM rows) + COST*t^T (accum MM)
      v     = w - eta_k * (q - mu)             (DVE; mu natural is constant)
      W     = proj_simplex(v) via 1 warm-started Newton round
              (theta state carried across iterations; fused relu+sum on ACT,
               fused mask+count on DVE)

The simplex projection is Newton on phi(t) = sum(relu(v - t)) - 1
(Michelot's method), warm-started so a single round per outer iteration
suffices (validated host-side end to end vs the sort-based reference).
"""
import os

import numpy as np

import concourse.bass as bass
import concourse.bass_utils as bass_utils
import concourse.mybir as mybir
import concourse.tile as tile
from concourse.bass_utils import run_bass_kernel_spmd
from concourse.vector_clock import ScopedClock



# ---------------------------------------------------------------------------
# Workaround for this container's walrus build, which only accepts a single
# sync-wait per instruction. Two pieces:
#   1. TileContext tail drain: spread its aggregated waits across extra
#      single-wait Drain instructions (sem-ge waits commute).
#   2. General post-pass: hoist excess waits from any instruction onto
#      injected single-wait NoOps on the same engine immediately before it
#      (per-engine program order preserved -> semantics preserved).
# ---------------------------------------------------------------------------


def _patched_drain_and_barrier(self, tick_clock, wait_clock):
    drain_inst = self.nc.sync.drain()
    wait_clock.add_sem_waits(
        drain_inst.ins, ScopedClock({None: tick_clock.global_clock})
    )
    si = drain_inst.ins.sync_info
    waits = list(si.on_wait or []) if si is not None else []
    if len(waits) > 1:
        drain_inst.ins.sync_info = mybir.SyncInfo(
            on_wait=[waits[0]], on_update=list(si.on_update or [])
        )
        for w in waits[1:]:
            extra = self.nc.sync.drain()
            extra.ins.sync_info = mybir.SyncInfo(on_wait=[w], on_update=[])
    self.nc.all_engine_barrier()
    assert self.sems is not None
    popped = self.nc._tile_sem_poison_stack.pop()
    assert popped is self._sem_poison
    self.nc.clear_and_free_semaphores(list(self.sems.allocated().values()))
    self.nc.all_engine_barrier()


tile.TileContext._drain_and_barrier = _patched_drain_and_barrier


def _legalize_sync_waits(nc, max_waits=1):
    n_split = 0
    for f in nc.m.functions:
        for b in f.blocks:
            il = b.instructions
            i = 0
            while i < len(il):
                inst = il[i]
                si = inst.sync_info
                if si is None:
                    i += 1
                    continue
                waits = list(si.on_wait or [])
                if len(waits) > max_waits:
                    keep = waits[:max_waits]
                    excess = waits[max_waits:]
                    inst.sync_info = mybir.SyncInfo(
                        on_wait=keep, on_update=list(si.on_update or [])
                    )
                    for w in excess:
                        nop = mybir.InstNoOp(
                            name=nc.get_next_instruction_name(),
                            engine=inst.engine,
                            ins=[],
                            outs=[],
                            sync_info=mybir.SyncInfo(on_wait=[w], on_update=[]),
                        )
                        nc.register_instruction(nop)
                        il.insert(i, nop)
                        i += 1
                        n_split += 1
                i += 1
    return n_split


# ---------------------------------------------------------------------------
# Problem constants (hardcoded per the task contract).
# ---------------------------------------------------------------------------
GAMMA = 5.0
COST = 1e-3
ITERS = int(os.environ.get("BASS_MPO_ITERS", "200"))
# Timing-only mode: constant step size so the eta table needn't cover ITERS
# columns; identical instruction mix, lets ITERS be amplified arbitrarily.
FIXED_ETA = os.environ.get("BASS_MPO_FIXED_ETA", "0") == "1"
# Timing-only amplification: repeat the full solve (re-init + ITERS + epilogue)
# OUTER times in an outer hardware loop. Output is unchanged (each rep
# recomputes the same result from scratch).
OUTER = int(os.environ.get("BASS_MPO_OUTER", "1"))
# Solver iterations per hardware-loop iteration. Each hardware For_i
# iteration ends in an all-engine barrier (semaphore reset block), a full
# pipeline drain; unrolling amortizes it. Must be even (A/B buffer parity)
# and divide ITERS.
UNROLL = int(os.environ.get("BASS_MPO_UNROLL", "4"))
ETA0 = 0.02
NEWTON_ROUNDS = 1

N_CORES = 8
B, H, N = 128, 12, 128
BC = B // N_CORES          # batches per core
V = BC * H                 # QP instances per core (= 192)

F32 = mybir.dt.float32
F16 = mybir.dt.float16
BF16 = mybir.dt.bfloat16
AF = mybir.ActivationFunctionType
OP = mybir.AluOpType


def _build_nc():
    nc = bass.Bass("TRN2", target_bir_lowering=False, debug=False)

    # L^T per instance, pre-scaled by sqrt(2*GAMMA), bf16: Sigma2G block v
    # is then one matmul LwT_v.T @ LwT_v with no on-chip transpose/scale.
    LwT = nc.dram_tensor("LwT", [V * N, N], BF16, kind="ExternalInput")
    NMU_T16 = nc.dram_tensor("NMU_T16", [N, V], F16, kind="ExternalInput")
    WPREV_T = nc.dram_tensor("WPREV_T", [N, BC], F32, kind="ExternalInput")
    NETA_W = 1 if FIXED_ETA else max(ITERS // UNROLL, 1)
    # Step-size tables for the unrolled loop: table u holds eta at solver
    # iterations UNROLL*i + u; the extra P table holds eta_{UNROLL*i - 1}
    # for the hoisted projection (col 0 unused: the k=0 hoisted projection
    # multiplies an exact zero).
    NEGETA_U = [
        nc.dram_tensor(f"NEGETA_U{u}", [N, NETA_W], F32, kind="ExternalInput")
        for u in range(UNROLL)
    ]
    NEGETA_P = nc.dram_tensor("NEGETA_P", [N, NETA_W], F32, kind="ExternalInput")
    IDT = nc.dram_tensor("IDT", [N, N], F32, kind="ExternalInput")
    IDTC16 = nc.dram_tensor("IDTC16", [N, N], F16, kind="ExternalInput")
    IDT16 = nc.dram_tensor("IDT16", [N, N], F16, kind="ExternalInput")
    WOUT = nc.dram_tensor("WOUT", [V, N], F32, kind="ExternalOutput")

    with tile.TileContext(nc) as tc:
        with tc.tile_pool(name="pers", bufs=1) as pers:
            idt = pers.tile([N, N], F32, tag="idt")
            nc.sync.dma_start(idt[:], IDT.ap())
            idc16 = pers.tile([N, N], F16, tag="idc16")
            nc.sync.dma_start(idc16[:], IDTC16.ap())
            idt16 = pers.tile([N, N], F16, tag="idt16")
            nc.sync.dma_start(idt16[:], IDT16.ap())
            nmu16 = pers.tile([N, V], F16, tag="nmu16")
            nc.sync.dma_start(nmu16[:], NMU_T16.ap())
            wprev = pers.tile([N, BC], F32, tag="wprev")
            nc.sync.dma_start(wprev[:], WPREV_T.ap())
            neta_u = []
            for u in range(UNROLL):
                t = pers.tile([N, NETA_W], F32, tag=f"neta_u{u}")
                nc.sync.dma_start(t[:], NEGETA_U[u].ap())
                neta_u.append(t)
            neta_p = pers.tile([N, NETA_W], F32, tag="neta_p")
            nc.sync.dma_start(neta_p[:], NEGETA_P.ap())

            sig16 = pers.tile([N, V * N], F16, tag="sig16")

            w0 = pers.tile([128, N], F32, tag="w0")
            w1 = pers.tile([64, N], F32, tag="w1")
            nth0 = pers.tile([128, 1], F32, tag="nth0")
            nth1 = pers.tile([64, 1], F32, tag="nth1")

            wt16 = pers.tile([N, V], F16, tag="wt16")
            wt_sb = pers.tile([N, V], F32, tag="wt_sb")
            dT = pers.tile([N, V], F32, tag="dT")
            sT = pers.tile([N, V], F16, tag="sT")
            tT = pers.tile([N, V], F16, tag="tT")

            # ---------------- Sigma precompute ----------------
            with tc.tile_pool(name="pre_ps", bufs=1, space="PSUM") as pps, \
                 tc.tile_pool(name="lstage", bufs=8) as lsp:
                for v in range(V):
                    ltb = lsp.tile([N, N], BF16, tag="ltb")
                    nc.sync.dma_start(ltb[:], LwT.ap()[v * N:(v + 1) * N, :])
                    sig_ps = pps.tile([N, N], F32, tag="sig", bufs=6)
                    nc.tensor.matmul(
                        sig_ps[:], ltb[:], ltb[:], start=True, stop=True
                    )
                    # PSUM->SBUF fp16 casts alternate DVE/ACT so neither
                    # engine bottlenecks the precompute stream.
                    if v % 2 == 0:
                        nc.scalar.copy(sig16[:, v * N:(v + 1) * N], sig_ps[:])
                    else:
                        nc.vector.tensor_copy(
                            sig16[:, v * N:(v + 1) * N], sig_ps[:]
                        )

            # ---------------- iteration loop ----------------
            # Software-pipelined: the hardware For_i body covers TWO solver
            # iterations (sub-bodies A and B) with explicit PSUM double
            # buffers (hardware loops cannot rotate pool buffers). Two
            # hoists keep the PE warm across boundaries:
            #   - proj-G1 of iteration k runs early in iteration k+1, under
            #     k+1's G0 matvec stream. At k=0 the seed qps1B = -mu makes
            #     the hoisted projection an exact no-op (v = w_init).
            #   - tr(w0)+fp16 cast for iteration k+1 run in the middle of
            #     iteration k's G1 matvec stream, so the next G0 stream
            #     starts immediately.
            assert UNROLL % 2 == 0 and ITERS % UNROLL == 0 and ITERS >= UNROLL
            with tc.tile_pool(name="lps", bufs=1, space="PSUM") as lps, \
                 tc.tile_pool(name="scr", bufs=1) as scr:
                wtps0_a = lps.tile([128, 128], F32, tag="wtps0_a")
                wtps0_b = lps.tile([128, 128], F32, tag="wtps0_b")
                wtps1 = lps.tile([128, 64], F32, tag="wtps1")
                qps0 = lps.tile([128, N], F32, tag="qps0")
                qps1_a = lps.tile([64, N], F32, tag="qps1_a")
                qps1_b = lps.tile([64, N], F32, tag="qps1_b")
                wt16g0 = pers.tile([N, 128], F16, tag="wt16g0")
                wt16g1 = pers.tile([N, 64], F16, tag="wt16g1")
                z16 = pers.tile([N, 64], F16, tag="z16")

                # 192 swapped matvecs, natural-layout output. Per 32-col
                # array strip, descending weight width: the matmul for
                # in-strip index r writes partitions [32j, 32j+r]; later
                # (narrower) matmuls overwrite the rows below, so partition
                # 32j+r keeps instance 32j+r's row.
                def _matvecs(qps, wtg, vbase, strips, rhi, rlo):
                    for r in range(rhi, rlo - 1, -1):
                        for j in strips:
                            v = vbase + 32 * j + r
                            nc.tensor.matmul(
                                qps[32 * j:32 * j + r + 1, :],
                                wtg[:, 32 * j:32 * j + r + 1],
                                sig16[:, v * N:(v + 1) * N],
                                start=True, stop=True,
                                tile_position=(0, 32 * j),
                            )

                def _update_proj(wt, nth, qps, pn, eta_ap):
                    vv = scr.tile([pn, N], F32, tag=f"v{pn}")
                    nc.vector.scalar_tensor_tensor(
                        vv[:], qps[:], eta_ap, wt[:],
                        op0=OP.mult, op1=OP.add,
                    )
                    for _ in range(NEWTON_ROUNDS):
                        rel = scr.tile([pn, N], F32, tag=f"rel{pn}")
                        sumr = scr.tile([pn, 1], F32, tag=f"sumr{pn}")
                        nc.scalar.activation(
                            rel[:], vv[:], AF.Relu,
                            bias=nth[:], scale=1.0, accum_out=sumr[:],
                        )
                        th = scr.tile([pn, 1], F32, tag=f"th{pn}")
                        nc.vector.tensor_scalar_mul(th[:], nth[:], -1.0)
                        # out = (v > theta); accum op1=add -> count
                        msk = scr.tile([pn, N], F32, tag=f"msk{pn}")
                        cnt = scr.tile([pn, 1], F32, tag=f"cnt{pn}")
                        nc.vector.tensor_scalar(
                            msk[:], vv[:], th[:], None,
                            op0=OP.is_gt, op1=OP.add, accum_out=cnt[:],
                        )
                        nc.vector.tensor_scalar_max(cnt[:], cnt[:], 1.0)
                        inv = scr.tile([pn, 1], F32, tag=f"inv{pn}")
                        nc.vector.reciprocal(inv[:], cnt[:])
                        dlt = scr.tile([pn, 1], F32, tag=f"dlt{pn}")
                        nc.vector.tensor_scalar(
                            dlt[:], sumr[:], -1.0, inv[:],
                            op0=OP.add, op1=OP.mult,
                        )
                        nc.vector.tensor_scalar_sub(nth[:], nth[:], dlt[:])
                    nc.scalar.activation(
                        wt[:], vv[:], AF.Relu, bias=nth[:], scale=1.0
                    )

                def _sub_body(wtps0_in, wtps0_out, qps1_prev, qps1_cur,
                              eta_g0, eta_g1prev):
                    # DVE head: G0 diffs from the pre-transposed W; the G0
                    # halves of sign/t are computed early so the COST-fold
                    # matmul below never waits on the G1 transpose.
                    nc.vector.tensor_copy(wt_sb[:, 0:128], wtps0_in[:])
                    nc.vector.tensor_sub(
                        dT[:, 0:BC], wt_sb[:, 0:BC], wprev[:]
                    )
                    nc.vector.tensor_sub(
                        dT[:, BC:128], wt_sb[:, BC:128], wt_sb[:, 0:128 - BC]
                    )
                    nc.scalar.sign(sT[:, 0:128], dT[:, 0:128])
                    nc.vector.tensor_sub(
                        tT[:, 0:128 - BC], sT[:, 0:128 - BC], sT[:, BC:128]
                    )
                    # PE: G0 stream, strips 0-2 (wt16g0 was cast in the
                    # previous sub-body; starts with no boundary stall).
                    _matvecs(qps0, wt16g0, 0, [0, 1, 2], 31, 0)
                    # Hoisted: previous iteration's G1 projection.
                    _update_proj(w1, nth1, qps1_prev, 64, eta_g1prev)
                    # PE: W_T(G1) for THIS iteration + casts + G1 diffs.
                    nc.tensor.transpose(wtps1[:], w1[:], idt[0:64, 0:64])
                    nc.scalar.copy(wt16g1[:], wtps1[:])
                    nc.vector.tensor_copy(wt_sb[:, 128:192], wtps1[:])
                    nc.vector.tensor_sub(
                        dT[:, 128:V], wt_sb[:, 128:V],
                        wt_sb[:, 128 - BC:V - BC],
                    )
                    nc.scalar.sign(sT[:, 128:V], dT[:, 128:V])
                    nc.vector.tensor_sub(
                        tT[:, 128 - BC:V - BC], sT[:, 128 - BC:V - BC],
                        sT[:, 128:V],
                    )
                    nc.vector.tensor_copy(tT[:, V - BC:V], sT[:, V - BC:V])
                    # PE: G0 stream strip 3, then fold COST*t^T and -mu^T.
                    _matvecs(qps0, wt16g0, 0, [3], 31, 0)
                    nc.tensor.matmul(
                        qps0[:, :], tT[:, 0:128], idc16[:, :],
                        start=False, stop=False, skip_group_check=True,
                    )
                    nc.tensor.matmul(
                        qps0[:, :], nmu16[:, 0:128], idt16[:, :],
                        start=False, stop=True, skip_group_check=True,
                    )
                    # G0 projection overlaps the G1 stream below.
                    _update_proj(w0, nth0, qps0, 128, eta_g0)
                    # PE: G1 stream (first 3/4), then hoisted tr(w0') +
                    # cast for the NEXT iteration, then the tail.
                    _matvecs(qps1_cur, wt16g1, 128, [0, 1], 31, 8)
                    nc.tensor.transpose(wtps0_out[:], w0[:], idt[:])
                    nc.scalar.copy(wt16g0[:], wtps0_out[:])
                    _matvecs(qps1_cur, wt16g1, 128, [0, 1], 7, 0)
                    nc.tensor.matmul(
                        qps1_cur[:, :], tT[:, 128:192], idc16[:, :],
                        start=False, stop=False, skip_group_check=True,
                    )
                    nc.tensor.matmul(
                        qps1_cur[:, :], nmu16[:, 128:192], idt16[:, :],
                        start=False, stop=True, skip_group_check=True,
                    )
                    # (G1 projection is hoisted into the next sub-body.)

                def _eta(tab, pn, idx):
                    if FIXED_ETA:
                        return tab[0:pn, 0:1]
                    return tab[0:pn, idx]

                import contextlib

                outer_cm = (
                    tc.For_i(0, OUTER, 1)
                    if OUTER > 1 else contextlib.nullcontext()
                )
                with outer_cm:
                    # State init + prologue: W_T(G0) for iteration 0; seed
                    # so the first hoisted G1 projection is an exact no-op.
                    nc.gpsimd.memset(w0[:], 1.0 / N)
                    nc.gpsimd.memset(w1[:], 1.0 / N)
                    nc.gpsimd.memset(nth0[:], 0.0)
                    nc.gpsimd.memset(nth1[:], 0.0)
                    nc.tensor.transpose(wtps0_a[:], w0[:], idt[:])
                    nc.scalar.copy(wt16g0[:], wtps0_a[:])
                    # Zero-seed qps1_b (PE matmul with a zero stationary) so
                    # the k=0 hoisted G1 projection is v = w_init exactly.
                    nc.gpsimd.memset(z16[:], 0.0)
                    nc.tensor.matmul(
                        qps1_b[:, :], z16[:], idt16[:, :],
                        start=True, stop=True,
                    )

                    with tc.For_i(0, ITERS // UNROLL, 1,
                                  staggered_reset=True) as k:
                        for u in range(UNROLL):
                            # sub-body u: solver iteration UNROLL*k + u;
                            # A/B buffers alternate on u parity.
                            ab = (u % 2 == 0)
                            _sub_body(
                                wtps0_a if ab else wtps0_b,
                                wtps0_b if ab else wtps0_a,
                                qps1_b if ab else qps1_a,
                                qps1_a if ab else qps1_b,
                                _eta(neta_u[u], 128, bass.ds(k, 1)),
                                _eta(neta_p if u == 0 else neta_u[u - 1],
                                     64, bass.ds(k, 1)),
                            )

                    # Epilogue: last iteration's G1 projection.
                    _update_proj(
                        w1, nth1, qps1_b, 64,
                        _eta(neta_u[UNROLL - 1], 64,
                             slice(ITERS // UNROLL - 1, ITERS // UNROLL)),
                    )

                nc.sync.dma_start(WOUT.ap()[0:128, :], w0[:])
                nc.sync.dma_start(WOUT.ap()[128:192, :], w1[:])

    _legalize_sync_waits(nc)
    return nc


def kernel(mu, L, w_prev):
    mu = np.ascontiguousarray(np.asarray(mu, dtype=np.float32))
    L = np.ascontiguousarray(np.asarray(L, dtype=np.float32))
    w_prev = np.ascontiguousarray(np.asarray(w_prev, dtype=np.float32))

    neta_w = 1 if FIXED_ETA else max(ITERS // UNROLL, 1)
    eta = (ETA0 / np.sqrt(np.arange(1, ITERS + 1, dtype=np.float32))).astype(
        np.float32
    )

    def _tab(col):
        return np.ascontiguousarray(
            np.broadcast_to(col[None, :], (N, neta_w)).astype(np.float32)
        )

    if FIXED_ETA:
        neta_us = [_tab(-eta[:1]) for _ in range(UNROLL)]
        neta_p = _tab(-eta[:1])
    else:
        neta_us = [_tab(-eta[u::UNROLL]) for u in range(UNROLL)]
        last = -eta[UNROLL - 1::UNROLL]                  # eta_{UNROLL*i - 1} shifted
        neta_p = _tab(np.concatenate([last[:1], last[:-1]]))
    idt = np.eye(N, dtype=np.float32)
    idc16 = (COST * np.eye(N)).astype(np.float16)
    idt16 = np.eye(N, dtype=np.float16)

    in_maps = []
    for c in range(N_CORES):
        bs = slice(c * BC, (c + 1) * BC)
        # h-major instance order: v = h*BC + b_local; per-instance L^T,
        # pre-scaled so the on-chip matmul directly yields 2*GAMMA*Sigma.
        import ml_dtypes
        LwT_c = np.ascontiguousarray(
            (np.sqrt(2.0 * GAMMA) * L[bs].transpose(1, 0, 3, 2))
            .reshape(V * N, N).astype(ml_dtypes.bfloat16)
        )
        nmu_t16 = np.ascontiguousarray(
            (-mu[bs]).transpose(2, 1, 0).reshape(N, V).astype(np.float16)
        )
        wprev_c = np.ascontiguousarray(w_prev[bs].T)
        in_maps.append(
            {
                "LwT": LwT_c,
                "NMU_T16": nmu_t16,
                "WPREV_T": wprev_c,
                **{f"NEGETA_U{u}": neta_us[u] for u in range(UNROLL)},
                "NEGETA_P": neta_p,
                "IDT": idt,
                "IDTC16": idc16,
                "IDT16": idt16,
            }
        )

    nc = _build_nc()
    res = run_bass_kernel_spmd(nc, in_maps, core_ids=list(range(N_CORES)))

    out = np.empty((B, H, N), dtype=np.float32)
    for c in range(N_CORES):
        wout = res.results[c]["WOUT"]  # [V, N], v = h*BC + b_local
        out[c * BC:(c + 1) * BC] = wout.reshape(H, BC, N).transpose(1, 0, 2)
    return out
